# revision 18
# baseline (speedup 1.0000x reference)
"""Trainium2 Bass kernel for the dense RandLA-Net block.

Reference computation (per batch b, point n, K=16 neighbors):
    enc   = [center(3), npos(3), rel(3), dist(1)]            # 10 dims
    rp    = relu(enc @ W_rel + b_rel)                        # 64
    f     = [rp, nfeat]                                      # 128
    att   = softmax_k(f @ W_att)                             # 128
    agg   = sum_k f * att                                    # 128
    out   = relu(agg @ W_glob + b_glob)                      # 128

Sharding: 8 cores = 4 batches x 2 point-halves (8192 points/core).
Per core the 131072 (point, k) pairs are processed in 16 tiles of 512
points (8192 k-major columns each).  One SWDGE transposed dma_gather per
tile fetches, for each column, a 256-byte row of a packed DRAM table
(bf16: x features at rows 0:64, pos at rows 64:67) straight into the
f-layout [128, 8192] tile.  dist is computed in a k-on-partitions
[48, 512] layout (free-size 512), summed by one matmul, sqrt'd and
DMA'd back into row 67; rp is one 7-row matmul per 512-col chunk
(rel folded: Wc*center + Wn*npos + Wr*(npos-center) = (Wc-Wr)*center +
(Wn+Wr)*npos) whose relu overwrites rows 64:128.  The channel order is
f = [nfeat; rp] everywhere (W_att row+col permuted, W_glob row permuted
host-side).  Softmax-weighted sums over k are PSUM-accumulated identity
matmuls.
"""

import sys

import numpy as np

sys.path.insert(0, "/opt/trn_rl_repo")

import ml_dtypes

import concourse.bass as bass
import concourse.tile as tile
from concourse import mybir, bacc
from concourse.bass_utils import run_bass_kernel_spmd

F32 = mybir.dt.float32
BF16 = mybir.dt.bfloat16
I16 = mybir.dt.int16
AF = mybir.ActivationFunctionType
OP = mybir.AluOpType
BF = ml_dtypes.bfloat16

B, C_IN, N, K = 4, 64, 16384, 16
D_REL, C_MID, C_OUT = 64, 128, 128
NP = N // 2            # points per core
NT = 16                # tiles (point blocks of 512)
TP = NP // NT          # 512 points per tile
PKT = TP * K           # 8192 columns per tile
NCH = 16               # 512-col chunks per tile


def _ap3(t2d, n_idx):
    # [128, n] 2D AP -> [128, 1, n] 3D AP for dma_gather transpose out
    return bass.AP(tensor=t2d.tensor, offset=t2d.offset,
                   ap=[[t2d.ap[0][0], 128], [n_idx, 1], [1, n_idx]])


def _view(t, apl):
    return bass.AP(tensor=t.tensor, offset=t.offset, ap=apl)


def _build_kernel():
    nc = bacc.Bacc("TRN2", target_bir_lowering=False, num_swdge_queues=4)

    tabT = nc.dram_tensor("tabT", [N, 128], BF16, kind="ExternalInput")
    idxg = nc.dram_tensor("idxg", [128, NP], I16, kind="ExternalInput")
    posC = nc.dram_tensor("posC", [3, NP], BF16, kind="ExternalInput")
    w7 = nc.dram_tensor("w7", [128, 64], BF16, kind="ExternalInput")
    watt = nc.dram_tensor("watt", [128, 128], BF16, kind="ExternalInput")
    wglob = nc.dram_tensor("wglob", [128, 128], BF16, kind="ExternalInput")
    w48 = nc.dram_tensor("w48", [128, 16], BF16, kind="ExternalInput")
    ident = nc.dram_tensor("ident", [128, 128], BF16, kind="ExternalInput")
    brel = nc.dram_tensor("brel", [128, 1], F32, kind="ExternalInput")
    bglob = nc.dram_tensor("bglob", [128, 1], F32, kind="ExternalInput")
    outp = nc.dram_tensor("outp", [128, NP], F32, kind="ExternalOutput")

    with tile.TileContext(nc) as tc:
        with tc.tile_pool(name="persist", bufs=1) as pp:
            idx_sb = pp.tile([128, NP], I16)
            posC_sb = pp.tile([3, NP], BF16)
            cen48 = pp.tile([48, NP], BF16)
            w7_sb = pp.tile([128, 64], BF16)
            watt_sb = pp.tile([128, 128], BF16)
            wglob_sb = pp.tile([128, 128], BF16)
            w48_sb = pp.tile([128, 16], BF16)
            ident_sb = pp.tile([128, 128], BF16)
            brel_sb = pp.tile([128, 1], F32)
            bglob_sb = pp.tile([128, 1], F32)
            nc.sync.dma_start(out=idx_sb, in_=idxg.ap())
            nc.sync.dma_start(out=posC_sb, in_=posC.ap())
            nc.sync.dma_start(out=w7_sb, in_=w7.ap())
            nc.sync.dma_start(out=watt_sb, in_=watt.ap())
            nc.sync.dma_start(out=wglob_sb, in_=wglob.ap())
            nc.sync.dma_start(out=w48_sb, in_=w48.ap())
            nc.sync.dma_start(out=ident_sb, in_=ident.ap())
            nc.sync.dma_start(out=brel_sb, in_=brel.ap())
            nc.sync.dma_start(out=bglob_sb, in_=bglob.ap())
            # cen48[16j+k, p] = posC[j, p]  (center replicated over k)
            src = posC_sb[0:3, :]
            nc.sync.dma_start(
                out=cen48[0:48, :],
                in_=_view(src, [[src.ap[0][0], 3], [0, 16], [1, NP]]))

            with tc.tile_pool(name="g0pool", bufs=2) as g0p, \
                 tc.tile_pool(name="gpool", bufs=3) as gp, \
                 tc.tile_pool(name="epool", bufs=2) as ep, \
                 tc.tile_pool(name="spool", bufs=2) as sp, \
                 tc.tile_pool(name="mps", bufs=2, space="PSUM") as mpsum, \
                 tc.tile_pool(name="scps", bufs=2, space="PSUM") as scpsum, \
                 tc.tile_pool(name="accps", bufs=2, space="PSUM") as apsum:

                def gather(t):
                    # raw row-gather: g0[p, s, :] = tabT[idx[s*128+p], :]
                    # split over the 4 SWDGE queues so the rings drain in
                    # parallel (each ring is serviced at ~1 DMA engine rate)
                    g0 = g0p.tile([128, PKT], BF16, tag="g0")
                    NQ, SUB = 4, PKT // 4
                    for q in range(NQ):
                        g0v = g0[:, q * SUB:(q + 1) * SUB]
                        g03 = _view(g0v, [[g0v.ap[0][0], 128],
                                          [128, SUB // 128], [1, 128]])
                        nc.gpsimd.dma_gather(
                            out_ap=g03, in_ap=tabT.ap(),
                            idxs_ap=idx_sb[:, t * TP + q * (SUB // 16):
                                           t * TP + (q + 1) * (SUB // 16)],
                            num_idxs=SUB, num_idxs_reg=SUB, elem_size=128,
                            transpose=False, single_packet=False, queue_num=q)
                    return g0

                def transpose_geom(t, g0):
                    # xbar transpose into f-layout: G[c, s*128+p] = g0[p, s*128+c]
                    g = gp.tile([128, PKT], BF16, tag="G")
                    gv = g[:, :]
                    g3 = _view(gv, [[gv.ap[0][0], 128], [128, PKT // 128],
                                    [1, 128]])
                    nc.sync.dma_start_transpose(g3, g0[:, :])

                    # geometry: dist into G row 67, center into 68:71
                    cb_src = posC_sb[0:3, t * TP:(t + 1) * TP]
                    cb_dst = g[68:71, :]
                    nc.sync.dma_start(
                        out=_view(cb_dst, [[cb_dst.ap[0][0], 3], [TP, 16],
                                           [1, TP]]),
                        in_=_view(cb_src, [[cb_src.ap[0][0], 3], [0, 16],
                                           [1, TP]]))
                    n48 = sp.tile([48, TP], BF16, tag="n48")
                    np_src = g[64:67, :]
                    nc.sync.dma_start(
                        out=n48[0:48, :],
                        in_=_view(np_src, [[np_src.ap[0][0], 3], [TP, 16],
                                           [1, TP]]))
                    nc.vector.tensor_tensor(
                        out=n48, in0=n48,
                        in1=cen48[0:48, t * TP:(t + 1) * TP], op=OP.subtract)
                    nc.vector.tensor_mul(n48, n48, n48)
                    psd = mpsum.tile([128, TP], F32, tag="rp")
                    nc.tensor.matmul(psd[0:16, :], w48_sb[0:48, :],
                                     n48[0:48, :], start=True, stop=True)
                    dsb = sp.tile([16, TP], BF16, tag="dsb")
                    nc.scalar.activation(out=dsb[0:16, :], in_=psd[0:16, :],
                                         func=AF.Sqrt)
                    d_dst = g[67:68, :]
                    nc.sync.dma_start(
                        out=_view(d_dst, [[d_dst.ap[0][0], 1], [TP, 16],
                                          [1, TP]]),
                        in_=dsb[0:16, :])
                    return g

                def epilogue(t, ps_den, ps_num):
                    # 1/den = exp(-ln(den)) on the scalar tables
                    lnd = sp.tile([128, TP], F32, tag="lnd")
                    nc.scalar.activation(out=lnd, in_=ps_den, func=AF.Ln)
                    rcp = sp.tile([128, TP], F32, tag="rcp")
                    nc.scalar.activation(out=rcp, in_=lnd, func=AF.Exp,
                                         scale=-1.0)
                    agg = sp.tile([128, TP], BF16, tag="agg")
                    nc.vector.tensor_mul(agg, ps_num, rcp)
                    ps_o = mpsum.tile([128, TP], F32, tag="rp")
                    nc.tensor.matmul(ps_o, wglob_sb, agg, start=True, stop=True)
                    osb = sp.tile([128, TP], F32, tag="osb")
                    nc.scalar.activation(out=osb, in_=ps_o, func=AF.Relu,
                                         bias=bglob_sb, scale=1.0)
                    nc.sync.dma_start(out=outp.ap()[:, t * TP:(t + 1) * TP],
                                      in_=osb)

                g0_list = [gather(0), gather(1)]
                g_list = [transpose_geom(0, g0_list[0])]
                pend = None
                for t in range(NT):
                    if t + 2 < NT:
                        g0_list.append(gather(t + 2))
                    if t + 1 < NT:
                        g_list.append(transpose_geom(t + 1, g0_list[t + 1]))
                    g = g_list[t]

                    # ---- phase A: rp chunks (PE keeps w7 resident)
                    for cc in range(NCH):
                        cols = slice(cc * TP, (cc + 1) * TP)
                        ps_rp = mpsum.tile([128, TP], F32, tag="rp")
                        nc.tensor.matmul(ps_rp[64:128, :], w7_sb[64:71, :],
                                         g[64:71, cols], start=True, stop=True,
                                         tile_position=(64, 64))
                        if cc % 2 == 0:
                            nc.scalar.activation(out=g[64:128, cols],
                                                 in_=ps_rp[64:128, :],
                                                 func=AF.Relu,
                                                 bias=brel_sb[64:128, :],
                                                 scale=1.0)
                        else:
                            nc.vector.tensor_scalar(out=g[64:128, cols],
                                                    in0=ps_rp[64:128, :],
                                                    scalar1=brel_sb[64:128, :],
                                                    scalar2=0.0,
                                                    op0=OP.add, op1=OP.max)

                    # ---- phase B: attention scores / exp / f*e (f*e in-place in G)
                    eu = ep.tile([128, PKT], BF16, tag="eu")
                    for cc in range(NCH):
                        cols = slice(cc * TP, (cc + 1) * TP)
                        ps_s = scpsum.tile([128, TP], F32, tag="sc")
                        nc.tensor.matmul(ps_s, watt_sb, g[:, cols],
                                         start=True, stop=True)
                        nc.scalar.activation(out=eu[:, cols], in_=ps_s,
                                             func=AF.Exp)
                        nc.vector.tensor_mul(g[:, cols], g[:, cols],
                                             eu[:, cols])

                    # ---- phase C: accumulate num/den over k on PE
                    ps_den = apsum.tile([128, TP], F32, tag="den")
                    ps_num = apsum.tile([128, TP], F32, tag="num")
                    for cc in range(NCH):
                        nc.tensor.matmul(ps_den, ident_sb,
                                         eu[:, cc * TP:(cc + 1) * TP],
                                         start=(cc == 0), stop=(cc == NCH - 1),
                                         skip_group_check=True)
                        nc.tensor.matmul(ps_num, ident_sb,
                                         g[:, cc * TP:(cc + 1) * TP],
                                         start=(cc == 0), stop=(cc == NCH - 1),
                                         skip_group_check=True)

                    # previous tile's softmax epilogue lands here so its
                    # reciprocal overlaps this tile's accumulation on the PE
                    if pend is not None:
                        epilogue(*pend)
                    pend = (t, ps_den, ps_num)
                epilogue(*pend)
    nc.compile()
    return nc


_NC = None


def _get_nc():
    global _NC
    if _NC is None:
        _NC = _build_kernel()
    return _NC


_PERM = (np.arange(128) + 64) % 128


def _prep_core(core, x, pos, neigh, Wc, Wn, Wr, wd, W_att, W_glob, b_rel, b_glob):
    b = core // 2
    half = core % 2
    P0 = half * NP
    nb = neigh[b][P0:P0 + NP].astype(np.int64)      # [NP, K]

    # packed gather table: row n = [x[:, n] | pos[n] | 0pad]  (bf16)
    tabT = np.zeros((N, 128), dtype=BF)
    tabT[:, 0:64] = x[b].T.astype(BF)
    tabT[:, 64:67] = pos[b].astype(BF)

    # gather idx: tile t cols (k,i) -> nb[t*512+i, k]; wrapped 16 + replicated
    A = nb.reshape(NT, TP, K)                        # [t, i, k]
    V = A.transpose(0, 2, 1).reshape(NT, PKT)        # [t, col] col=k*512+i
    W16 = V.reshape(NT, TP, 16).transpose(0, 2, 1)   # [t, j, s]: idx s*16+j
    idxg = np.tile(W16.transpose(1, 0, 2).reshape(16, NP), (8, 1)).astype(np.int16)

    posCa = pos[b][P0:P0 + NP].T.astype(BF)          # [3, NP]

    w7v = np.zeros((128, 64), dtype=BF)
    w7v[64:67] = (Wn + Wr).astype(BF)
    w7v[67:68] = wd.astype(BF)
    w7v[68:71] = (Wc - Wr).astype(BF)

    w48 = np.zeros((128, 16), dtype=BF)
    for j in range(3):
        for k in range(16):
            w48[16 * j + k, k] = 1

    brel_full = np.zeros((128, 1), np.float32)
    brel_full[64:128, 0] = b_rel

    return {
        "tabT": tabT, "idxg": idxg, "posC": posCa,
        "w7": w7v,
        "watt": W_att[np.ix_(_PERM, _PERM)].astype(BF),
        "wglob": W_glob[_PERM, :].astype(BF),
        "w48": w48,
        "ident": np.eye(128, dtype=BF),
        "brel": brel_full,
        "bglob": b_glob.reshape(128, 1).astype(np.float32),
    }


def kernel(x, pos, neigh_idx, W_rel, b_rel, W_att, W_glob, b_glob, **kw):
    x = np.ascontiguousarray(np.asarray(x, dtype=np.float32))
    pos = np.ascontiguousarray(np.asarray(pos, dtype=np.float32))
    neigh = np.asarray(neigh_idx)
    W_rel = np.asarray(W_rel, dtype=np.float32)
    W_att = np.asarray(W_att, dtype=np.float32)
    W_glob = np.asarray(W_glob, dtype=np.float32)
    b_rel = np.asarray(b_rel, dtype=np.float32)
    b_glob = np.asarray(b_glob, dtype=np.float32)
    Wc, Wn, Wr, wd = W_rel[0:3], W_rel[3:6], W_rel[6:9], W_rel[9:10]

    nc = _get_nc()
    in_maps = [
        _prep_core(core, x, pos, neigh, Wc, Wn, Wr, wd, W_att, W_glob, b_rel, b_glob)
        for core in range(8)
    ]
    res = run_bass_kernel_spmd(nc, in_maps, core_ids=list(range(8)))
    out = np.zeros((B, C_OUT, N), np.float32)
    for core in range(8):
        b = core // 2
        P0 = (core % 2) * NP
        out[b, :, P0:P0 + NP] = res.results[core]["outp"]
    return out


# revision 19
# speedup vs baseline: 1.1311x; 1.1311x over previous
"""Trainium2 Bass kernel for the dense RandLA-Net block.

Reference computation (per batch b, point n, K=16 neighbors):
    enc   = [center(3), npos(3), rel(3), dist(1)]            # 10 dims
    rp    = relu(enc @ W_rel + b_rel)                        # 64
    f     = [rp, nfeat]                                      # 128
    att   = softmax_k(f @ W_att)                             # 128
    agg   = sum_k f * att                                    # 128
    out   = relu(agg @ W_glob + b_glob)                      # 128

Sharding: 8 cores = 4 batches x 2 point-halves (8192 points/core).
Per core the 131072 (point, k) pairs are processed in 16 tiles of 512
points (8192 k-major columns each).  One SWDGE transposed dma_gather per
tile fetches, for each column, a 256-byte row of a packed DRAM table
(bf16: x features at rows 0:64, pos at rows 64:67) straight into the
f-layout [128, 8192] tile.  dist is computed in a k-on-partitions
[48, 512] layout (free-size 512), summed by one matmul, sqrt'd and
DMA'd back into row 67; rp is one 7-row matmul per 512-col chunk
(rel folded: Wc*center + Wn*npos + Wr*(npos-center) = (Wc-Wr)*center +
(Wn+Wr)*npos) whose relu overwrites rows 64:128.  The channel order is
f = [nfeat; rp] everywhere (W_att row+col permuted, W_glob row permuted
host-side).  Softmax-weighted sums over k are PSUM-accumulated identity
matmuls.
"""

import sys

import numpy as np

sys.path.insert(0, "/opt/trn_rl_repo")

import ml_dtypes

import concourse.bass as bass
import concourse.tile as tile
from concourse import mybir, bacc
from concourse.bass_utils import run_bass_kernel_spmd

F32 = mybir.dt.float32
BF16 = mybir.dt.bfloat16
I16 = mybir.dt.int16
AF = mybir.ActivationFunctionType
OP = mybir.AluOpType
BF = ml_dtypes.bfloat16

B, C_IN, N, K = 4, 64, 16384, 16
D_REL, C_MID, C_OUT = 64, 128, 128
NP = N // 2            # points per core
NT = 16                # tiles (point blocks of 512)
TP = NP // NT          # 512 points per tile
PKT = TP * K           # 8192 columns per tile
NCH = 16               # 512-col chunks per tile


def _ap3(t2d, n_idx):
    # [128, n] 2D AP -> [128, 1, n] 3D AP for dma_gather transpose out
    return bass.AP(tensor=t2d.tensor, offset=t2d.offset,
                   ap=[[t2d.ap[0][0], 128], [n_idx, 1], [1, n_idx]])


def _view(t, apl):
    return bass.AP(tensor=t.tensor, offset=t.offset, ap=apl)


def _build_kernel():
    nc = bacc.Bacc("TRN2", target_bir_lowering=False, num_swdge_queues=4)

    tabT = nc.dram_tensor("tabT", [N, 128], BF16, kind="ExternalInput")
    idxg = nc.dram_tensor("idxg", [128, NP], I16, kind="ExternalInput")
    posC = nc.dram_tensor("posC", [3, NP], BF16, kind="ExternalInput")
    w7 = nc.dram_tensor("w7", [128, 64], BF16, kind="ExternalInput")
    watt = nc.dram_tensor("watt", [128, 128], BF16, kind="ExternalInput")
    wglob = nc.dram_tensor("wglob", [128, 128], BF16, kind="ExternalInput")
    w48 = nc.dram_tensor("w48", [128, 16], BF16, kind="ExternalInput")
    ident = nc.dram_tensor("ident", [128, 128], BF16, kind="ExternalInput")
    brel = nc.dram_tensor("brel", [128, 1], F32, kind="ExternalInput")
    bglob = nc.dram_tensor("bglob", [128, 1], F32, kind="ExternalInput")
    outp = nc.dram_tensor("outp", [128, NP], F32, kind="ExternalOutput")

    with tile.TileContext(nc) as tc:
        with tc.tile_pool(name="persist", bufs=1) as pp:
            idx_sb = pp.tile([128, NP], I16)
            posC_sb = pp.tile([3, NP], BF16)
            cen48 = pp.tile([48, NP], BF16)
            w7_sb = pp.tile([128, 64], BF16)
            watt_sb = pp.tile([128, 128], BF16)
            wglob_sb = pp.tile([128, 128], BF16)
            w48_sb = pp.tile([128, 16], BF16)
            ident_sb = pp.tile([128, 128], BF16)
            brel_sb = pp.tile([128, 1], F32)
            bglob_sb = pp.tile([128, 1], F32)
            nc.sync.dma_start(out=idx_sb, in_=idxg.ap())
            nc.sync.dma_start(out=posC_sb, in_=posC.ap())
            nc.sync.dma_start(out=w7_sb, in_=w7.ap())
            nc.sync.dma_start(out=watt_sb, in_=watt.ap())
            nc.sync.dma_start(out=wglob_sb, in_=wglob.ap())
            nc.sync.dma_start(out=w48_sb, in_=w48.ap())
            nc.sync.dma_start(out=ident_sb, in_=ident.ap())
            nc.sync.dma_start(out=brel_sb, in_=brel.ap())
            nc.sync.dma_start(out=bglob_sb, in_=bglob.ap())
            # cen48[16j+k, p] = posC[j, p]  (center replicated over k)
            src = posC_sb[0:3, :]
            nc.sync.dma_start(
                out=cen48[0:48, :],
                in_=_view(src, [[src.ap[0][0], 3], [0, 16], [1, NP]]))

            with tc.tile_pool(name="g0pool", bufs=2) as g0p, \
                 tc.tile_pool(name="gpool", bufs=3) as gp, \
                 tc.tile_pool(name="epool", bufs=2) as ep, \
                 tc.tile_pool(name="spool", bufs=2) as sp, \
                 tc.tile_pool(name="mps", bufs=2, space="PSUM") as mpsum, \
                 tc.tile_pool(name="scps", bufs=2, space="PSUM") as scpsum, \
                 tc.tile_pool(name="accps", bufs=2, space="PSUM") as apsum:

                def gather(t):
                    # raw row-gather: g0[p, s, :] = tabT[idx[s*128+p], :]
                    # split over the 4 SWDGE queues so the rings drain in
                    # parallel (each ring is serviced at ~1 DMA engine rate)
                    g0 = g0p.tile([128, PKT], BF16, tag="g0")
                    NQ, SUB = 4, PKT // 4
                    for q in range(NQ):
                        g0v = g0[:, q * SUB:(q + 1) * SUB]
                        g03 = _view(g0v, [[g0v.ap[0][0], 128],
                                          [128, SUB // 128], [1, 128]])
                        nc.gpsimd.dma_gather(
                            out_ap=g03, in_ap=tabT.ap(),
                            idxs_ap=idx_sb[:, t * TP + q * (SUB // 16):
                                           t * TP + (q + 1) * (SUB // 16)],
                            num_idxs=SUB, num_idxs_reg=SUB, elem_size=128,
                            transpose=False, single_packet=False, queue_num=q)
                    return g0

                def transpose_geom(t, g0):
                    # xbar transpose into f-layout: G[c, s*128+p] = g0[p, s*128+c]
                    g = gp.tile([128, PKT], BF16, tag="G")
                    gv = g[:, :]
                    g3 = _view(gv, [[gv.ap[0][0], 128], [128, PKT // 128],
                                    [1, 128]])
                    nc.sync.dma_start_transpose(g3, g0[:, :])

                    # geometry: dist into G row 67, center into 68:71
                    cb_src = posC_sb[0:3, t * TP:(t + 1) * TP]
                    cb_dst = g[68:71, :]
                    nc.sync.dma_start(
                        out=_view(cb_dst, [[cb_dst.ap[0][0], 3], [TP, 16],
                                           [1, TP]]),
                        in_=_view(cb_src, [[cb_src.ap[0][0], 3], [0, 16],
                                           [1, TP]]))
                    n48 = sp.tile([48, TP], BF16, tag="n48")
                    np_src = g[64:67, :]
                    nc.sync.dma_start(
                        out=n48[0:48, :],
                        in_=_view(np_src, [[np_src.ap[0][0], 3], [TP, 16],
                                           [1, TP]]))
                    nc.vector.tensor_tensor(
                        out=n48, in0=n48,
                        in1=cen48[0:48, t * TP:(t + 1) * TP], op=OP.subtract)
                    nc.vector.tensor_mul(n48, n48, n48)
                    psd = mpsum.tile([128, TP], F32, tag="rp")
                    nc.tensor.matmul(psd[0:16, :], w48_sb[0:48, :],
                                     n48[0:48, :], start=True, stop=True)
                    dsb = sp.tile([16, TP], BF16, tag="dsb")
                    nc.scalar.activation(out=dsb[0:16, :], in_=psd[0:16, :],
                                         func=AF.Sqrt)
                    d_dst = g[67:68, :]
                    nc.sync.dma_start(
                        out=_view(d_dst, [[d_dst.ap[0][0], 1], [TP, 16],
                                          [1, TP]]),
                        in_=dsb[0:16, :])
                    return g

                def epilogue(t, ps_den, ps_num):
                    rcp = sp.tile([128, TP], F32, tag="rcp")
                    nc.vector.reciprocal(rcp, ps_den)
                    agg = sp.tile([128, TP], BF16, tag="agg")
                    nc.vector.tensor_mul(agg, ps_num, rcp)
                    ps_o = mpsum.tile([128, TP], F32, tag="rp")
                    nc.tensor.matmul(ps_o, wglob_sb, agg, start=True, stop=True)
                    osb = sp.tile([128, TP], F32, tag="osb")
                    nc.scalar.activation(out=osb, in_=ps_o, func=AF.Relu,
                                         bias=bglob_sb, scale=1.0)
                    nc.sync.dma_start(out=outp.ap()[:, t * TP:(t + 1) * TP],
                                      in_=osb)

                g0_list = [gather(0), gather(1)]
                g_list = [transpose_geom(0, g0_list[0])]
                pend = None
                for t in range(NT):
                    if t + 2 < NT:
                        g0_list.append(gather(t + 2))
                    if t + 1 < NT:
                        g_list.append(transpose_geom(t + 1, g0_list[t + 1]))
                    g = g_list[t]

                    # ---- phase A: rp chunks (PE keeps w7 resident)
                    for cc in range(NCH):
                        cols = slice(cc * TP, (cc + 1) * TP)
                        ps_rp = mpsum.tile([128, TP], F32, tag="rp")
                        nc.tensor.matmul(ps_rp[64:128, :], w7_sb[64:71, :],
                                         g[64:71, cols], start=True, stop=True,
                                         tile_position=(64, 64))
                        if cc % 2 == 0:
                            nc.scalar.activation(out=g[64:128, cols],
                                                 in_=ps_rp[64:128, :],
                                                 func=AF.Relu,
                                                 bias=brel_sb[64:128, :],
                                                 scale=1.0)
                        else:
                            nc.vector.tensor_scalar(out=g[64:128, cols],
                                                    in0=ps_rp[64:128, :],
                                                    scalar1=brel_sb[64:128, :],
                                                    scalar2=0.0,
                                                    op0=OP.add, op1=OP.max)

                    # ---- phase B: attention scores / exp / f*e (f*e in-place in G)
                    eu = ep.tile([128, PKT], BF16, tag="eu")
                    for cc in range(NCH):
                        cols = slice(cc * TP, (cc + 1) * TP)
                        ps_s = scpsum.tile([128, TP], F32, tag="sc")
                        nc.tensor.matmul(ps_s, watt_sb, g[:, cols],
                                         start=True, stop=True)
                        nc.scalar.activation(out=eu[:, cols], in_=ps_s,
                                             func=AF.Exp)
                        nc.vector.tensor_mul(g[:, cols], g[:, cols],
                                             eu[:, cols])

                    # ---- phase C: accumulate num/den over k on PE
                    ps_den = apsum.tile([128, TP], F32, tag="den")
                    ps_num = apsum.tile([128, TP], F32, tag="num")
                    for cc in range(NCH):
                        nc.tensor.matmul(ps_den, ident_sb,
                                         eu[:, cc * TP:(cc + 1) * TP],
                                         start=(cc == 0), stop=(cc == NCH - 1),
                                         skip_group_check=True)
                        nc.tensor.matmul(ps_num, ident_sb,
                                         g[:, cc * TP:(cc + 1) * TP],
                                         start=(cc == 0), stop=(cc == NCH - 1),
                                         skip_group_check=True)

                    # previous tile's softmax epilogue lands here so its
                    # reciprocal overlaps this tile's accumulation on the PE
                    if pend is not None:
                        epilogue(*pend)
                    pend = (t, ps_den, ps_num)
                epilogue(*pend)
    nc.compile()
    return nc


_NC = None


def _get_nc():
    global _NC
    if _NC is None:
        _NC = _build_kernel()
    return _NC


_PERM = (np.arange(128) + 64) % 128


def _prep_core(core, x, pos, neigh, Wc, Wn, Wr, wd, W_att, W_glob, b_rel, b_glob):
    b = core // 2
    half = core % 2
    P0 = half * NP
    nb = neigh[b][P0:P0 + NP].astype(np.int64)      # [NP, K]

    # packed gather table: row n = [x[:, n] | pos[n] | 0pad]  (bf16)
    tabT = np.zeros((N, 128), dtype=BF)
    tabT[:, 0:64] = x[b].T.astype(BF)
    tabT[:, 64:67] = pos[b].astype(BF)

    # gather idx: tile t cols (k,i) -> nb[t*512+i, k]; wrapped 16 + replicated
    A = nb.reshape(NT, TP, K)                        # [t, i, k]
    V = A.transpose(0, 2, 1).reshape(NT, PKT)        # [t, col] col=k*512+i
    W16 = V.reshape(NT, TP, 16).transpose(0, 2, 1)   # [t, j, s]: idx s*16+j
    idxg = np.tile(W16.transpose(1, 0, 2).reshape(16, NP), (8, 1)).astype(np.int16)

    posCa = pos[b][P0:P0 + NP].T.astype(BF)          # [3, NP]

    w7v = np.zeros((128, 64), dtype=BF)
    w7v[64:67] = (Wn + Wr).astype(BF)
    w7v[67:68] = wd.astype(BF)
    w7v[68:71] = (Wc - Wr).astype(BF)

    w48 = np.zeros((128, 16), dtype=BF)
    for j in range(3):
        for k in range(16):
            w48[16 * j + k, k] = 1

    brel_full = np.zeros((128, 1), np.float32)
    brel_full[64:128, 0] = b_rel

    return {
        "tabT": tabT, "idxg": idxg, "posC": posCa,
        "w7": w7v,
        "watt": W_att[np.ix_(_PERM, _PERM)].astype(BF),
        "wglob": W_glob[_PERM, :].astype(BF),
        "w48": w48,
        "ident": np.eye(128, dtype=BF),
        "brel": brel_full,
        "bglob": b_glob.reshape(128, 1).astype(np.float32),
    }


def kernel(x, pos, neigh_idx, W_rel, b_rel, W_att, W_glob, b_glob, **kw):
    x = np.ascontiguousarray(np.asarray(x, dtype=np.float32))
    pos = np.ascontiguousarray(np.asarray(pos, dtype=np.float32))
    neigh = np.asarray(neigh_idx)
    W_rel = np.asarray(W_rel, dtype=np.float32)
    W_att = np.asarray(W_att, dtype=np.float32)
    W_glob = np.asarray(W_glob, dtype=np.float32)
    b_rel = np.asarray(b_rel, dtype=np.float32)
    b_glob = np.asarray(b_glob, dtype=np.float32)
    Wc, Wn, Wr, wd = W_rel[0:3], W_rel[3:6], W_rel[6:9], W_rel[9:10]

    nc = _get_nc()
    in_maps = [
        _prep_core(core, x, pos, neigh, Wc, Wn, Wr, wd, W_att, W_glob, b_rel, b_glob)
        for core in range(8)
    ]
    res = run_bass_kernel_spmd(nc, in_maps, core_ids=list(range(8)))
    out = np.zeros((B, C_OUT, N), np.float32)
    for core in range(8):
        b = core // 2
        P0 = (core % 2) * NP
        out[b, :, P0:P0 + NP] = res.results[core]["outp"]
    return out


# revision 20
# speedup vs baseline: 1.2345x; 1.0914x over previous
"""Trainium2 Bass kernel for the dense RandLA-Net block.

Reference computation (per batch b, point n, K=16 neighbors):
    enc   = [center(3), npos(3), rel(3), dist(1)]            # 10 dims
    rp    = relu(enc @ W_rel + b_rel)                        # 64
    f     = [rp, nfeat]                                      # 128
    att   = softmax_k(f @ W_att)                             # 128
    agg   = sum_k f * att                                    # 128
    out   = relu(agg @ W_glob + b_glob)                      # 128

Sharding: 8 cores = 4 batches x 2 point-halves (8192 points/core).
Per core the 131072 (point, k) pairs are processed in 16 tiles of 512
points (8192 k-major columns each).  One SWDGE transposed dma_gather per
tile fetches, for each column, a 256-byte row of a packed DRAM table
(bf16: x features at rows 0:64, pos at rows 64:67) straight into the
f-layout [128, 8192] tile.  dist is computed in a k-on-partitions
[48, 512] layout (free-size 512), summed by one matmul, sqrt'd and
DMA'd back into row 67; rp is one 7-row matmul per 512-col chunk
(rel folded: Wc*center + Wn*npos + Wr*(npos-center) = (Wc-Wr)*center +
(Wn+Wr)*npos) whose relu overwrites rows 64:128.  The channel order is
f = [nfeat; rp] everywhere (W_att row+col permuted, W_glob row permuted
host-side).  Softmax-weighted sums over k are PSUM-accumulated identity
matmuls.
"""

import sys

import numpy as np

sys.path.insert(0, "/opt/trn_rl_repo")

import ml_dtypes

import concourse.bass as bass
import concourse.tile as tile
from concourse import mybir, bacc
from concourse.bass_utils import run_bass_kernel_spmd

F32 = mybir.dt.float32
BF16 = mybir.dt.bfloat16
I16 = mybir.dt.int16
AF = mybir.ActivationFunctionType
OP = mybir.AluOpType
BF = ml_dtypes.bfloat16

B, C_IN, N, K = 4, 64, 16384, 16
D_REL, C_MID, C_OUT = 64, 128, 128
NP = N // 2            # points per core
NT = 16                # tiles (point blocks of 512)
TP = NP // NT          # 512 points per tile
PKT = TP * K           # 8192 columns per tile
NCH = 16               # 512-col chunks per tile


def _ap3(t2d, n_idx):
    # [128, n] 2D AP -> [128, 1, n] 3D AP for dma_gather transpose out
    return bass.AP(tensor=t2d.tensor, offset=t2d.offset,
                   ap=[[t2d.ap[0][0], 128], [n_idx, 1], [1, n_idx]])


def _view(t, apl):
    return bass.AP(tensor=t.tensor, offset=t.offset, ap=apl)


def _build_kernel():
    nc = bacc.Bacc("TRN2", target_bir_lowering=False, num_swdge_queues=4)

    tabT = nc.dram_tensor("tabT", [N, 128], BF16, kind="ExternalInput")
    idxg = nc.dram_tensor("idxg", [128, NP], I16, kind="ExternalInput")
    posC = nc.dram_tensor("posC", [3, NP], BF16, kind="ExternalInput")
    w7 = nc.dram_tensor("w7", [128, 64], BF16, kind="ExternalInput")
    watt = nc.dram_tensor("watt", [128, 128], BF16, kind="ExternalInput")
    wglob = nc.dram_tensor("wglob", [128, 128], BF16, kind="ExternalInput")
    w48 = nc.dram_tensor("w48", [128, 16], BF16, kind="ExternalInput")
    ident = nc.dram_tensor("ident", [128, 128], BF16, kind="ExternalInput")
    brel = nc.dram_tensor("brel", [128, 1], F32, kind="ExternalInput")
    bglob = nc.dram_tensor("bglob", [128, 1], F32, kind="ExternalInput")
    outp = nc.dram_tensor("outp", [128, NP], F32, kind="ExternalOutput")

    with tile.TileContext(nc) as tc:
        with tc.tile_pool(name="persist", bufs=1) as pp:
            idx_sb = pp.tile([128, NP], I16)
            posC_sb = pp.tile([3, NP], BF16)
            cen48 = pp.tile([48, NP], BF16)
            w7_sb = pp.tile([128, 64], BF16)
            watt_sb = pp.tile([128, 128], BF16)
            wglob_sb = pp.tile([128, 128], BF16)
            w48_sb = pp.tile([128, 16], BF16)
            ident_sb = pp.tile([128, 128], BF16)
            brel_sb = pp.tile([128, 1], F32)
            bglob_sb = pp.tile([128, 1], F32)
            nc.sync.dma_start(out=idx_sb, in_=idxg.ap())
            nc.sync.dma_start(out=posC_sb, in_=posC.ap())
            nc.sync.dma_start(out=w7_sb, in_=w7.ap())
            nc.sync.dma_start(out=watt_sb, in_=watt.ap())
            nc.sync.dma_start(out=wglob_sb, in_=wglob.ap())
            nc.sync.dma_start(out=w48_sb, in_=w48.ap())
            nc.sync.dma_start(out=ident_sb, in_=ident.ap())
            nc.sync.dma_start(out=brel_sb, in_=brel.ap())
            nc.sync.dma_start(out=bglob_sb, in_=bglob.ap())
            # cen48[16j+k, p] = posC[j, p]  (center replicated over k)
            src = posC_sb[0:3, :]
            nc.sync.dma_start(
                out=cen48[0:48, :],
                in_=_view(src, [[src.ap[0][0], 3], [0, 16], [1, NP]]))

            with tc.tile_pool(name="g0pool", bufs=2) as g0p, \
                 tc.tile_pool(name="gpool", bufs=3) as gp, \
                 tc.tile_pool(name="epool", bufs=2) as ep, \
                 tc.tile_pool(name="spool", bufs=2) as sp, \
                 tc.tile_pool(name="mps", bufs=2, space="PSUM") as mpsum, \
                 tc.tile_pool(name="scps", bufs=2, space="PSUM") as scpsum, \
                 tc.tile_pool(name="accps", bufs=2, space="PSUM") as apsum:

                def gather(t):
                    # raw row-gather: g0[p, s, :] = tabT[idx[s*128+p], :]
                    # split over the 4 SWDGE queues so the rings drain in
                    # parallel (each ring is serviced at ~1 DMA engine rate)
                    g0 = g0p.tile([128, PKT], BF16, tag="g0")
                    NQ, SUB = 4, PKT // 4
                    for q in range(NQ):
                        g0v = g0[:, q * SUB:(q + 1) * SUB]
                        g03 = _view(g0v, [[g0v.ap[0][0], 128],
                                          [128, SUB // 128], [1, 128]])
                        nc.gpsimd.dma_gather(
                            out_ap=g03, in_ap=tabT.ap(),
                            idxs_ap=idx_sb[:, t * TP + q * (SUB // 16):
                                           t * TP + (q + 1) * (SUB // 16)],
                            num_idxs=SUB, num_idxs_reg=SUB, elem_size=128,
                            transpose=False, single_packet=False, queue_num=q)
                    return g0

                def transpose_geom(t, g0):
                    # xbar transpose into f-layout: G[c, s*128+p] = g0[p, s*128+c]
                    g = gp.tile([128, PKT], BF16, tag="G")
                    gv = g[:, :]
                    g3 = _view(gv, [[gv.ap[0][0], 128], [128, PKT // 128],
                                    [1, 128]])
                    nc.sync.dma_start_transpose(g3, g0[:, :])

                    # geometry: dist into G row 67, center into 68:71
                    cb_src = posC_sb[0:3, t * TP:(t + 1) * TP]
                    cb_dst = g[68:71, :]
                    nc.sync.dma_start(
                        out=_view(cb_dst, [[cb_dst.ap[0][0], 3], [TP, 16],
                                           [1, TP]]),
                        in_=_view(cb_src, [[cb_src.ap[0][0], 3], [0, 16],
                                           [1, TP]]))
                    n48 = sp.tile([48, TP], BF16, tag="n48")
                    np_src = g[64:67, :]
                    nc.sync.dma_start(
                        out=n48[0:48, :],
                        in_=_view(np_src, [[np_src.ap[0][0], 3], [TP, 16],
                                           [1, TP]]))
                    nc.vector.tensor_tensor(
                        out=n48, in0=n48,
                        in1=cen48[0:48, t * TP:(t + 1) * TP], op=OP.subtract)
                    nc.vector.tensor_mul(n48, n48, n48)
                    psd = scpsum.tile([128, TP], F32, tag="sc")
                    nc.tensor.matmul(psd[0:16, :], w48_sb[0:48, :],
                                     n48[0:48, :], start=True, stop=True)
                    dsb = sp.tile([16, TP], BF16, tag="dsb")
                    nc.scalar.activation(out=dsb[0:16, :], in_=psd[0:16, :],
                                         func=AF.Sqrt)
                    d_dst = g[67:68, :]
                    nc.sync.dma_start(
                        out=_view(d_dst, [[d_dst.ap[0][0], 1], [TP, 16],
                                          [1, TP]]),
                        in_=dsb[0:16, :])
                    return g

                def epilogue(t, ps_den, ps_num):
                    rcp = sp.tile([128, TP], F32, tag="rcp")
                    nc.vector.reciprocal(rcp, ps_den)
                    agg = sp.tile([128, TP], BF16, tag="agg")
                    nc.vector.tensor_mul(agg, ps_num, rcp)
                    ps_o = mpsum.tile([128, TP], F32, tag="rp")
                    nc.tensor.matmul(ps_o, wglob_sb, agg, start=True, stop=True)
                    osb = sp.tile([128, TP], F32, tag="osb")
                    nc.scalar.activation(out=osb, in_=ps_o, func=AF.Relu,
                                         bias=bglob_sb, scale=1.0)
                    nc.sync.dma_start(out=outp.ap()[:, t * TP:(t + 1) * TP],
                                      in_=osb)

                g0_list = [gather(0), gather(1)]
                g_list = [transpose_geom(0, g0_list[0])]
                pend = None
                for t in range(NT):
                    if t + 2 < NT:
                        g0_list.append(gather(t + 2))
                    if t + 1 < NT:
                        g_list.append(transpose_geom(t + 1, g0_list[t + 1]))
                    g = g_list[t]

                    # ---- phase A: rp chunks (PE keeps w7 resident)
                    for cc in range(NCH):
                        cols = slice(cc * TP, (cc + 1) * TP)
                        ps_rp = mpsum.tile([128, TP], F32, tag="rp")
                        nc.tensor.matmul(ps_rp[64:128, :], w7_sb[64:71, :],
                                         g[64:71, cols], start=True, stop=True,
                                         tile_position=(64, 64))
                        if cc % 2 == 0:
                            nc.scalar.activation(out=g[64:128, cols],
                                                 in_=ps_rp[64:128, :],
                                                 func=AF.Relu,
                                                 bias=brel_sb[64:128, :],
                                                 scale=1.0)
                        else:
                            nc.vector.tensor_scalar(out=g[64:128, cols],
                                                    in0=ps_rp[64:128, :],
                                                    scalar1=brel_sb[64:128, :],
                                                    scalar2=0.0,
                                                    op0=OP.add, op1=OP.max)

                    # ---- phase B: attention scores / exp / f*e (f*e in-place in G)
                    eu = ep.tile([128, PKT], BF16, tag="eu")
                    for cc in range(NCH):
                        cols = slice(cc * TP, (cc + 1) * TP)
                        ps_s = scpsum.tile([128, TP], F32, tag="sc")
                        nc.tensor.matmul(ps_s, watt_sb, g[:, cols],
                                         start=True, stop=True)
                        nc.scalar.activation(out=eu[:, cols], in_=ps_s,
                                             func=AF.Exp)
                        nc.vector.tensor_mul(g[:, cols], g[:, cols],
                                             eu[:, cols])

                    # ---- phase C: accumulate num/den over k on PE
                    ps_den = apsum.tile([128, TP], F32, tag="den")
                    ps_num = apsum.tile([128, TP], F32, tag="num")
                    for cc in range(NCH):
                        nc.tensor.matmul(ps_den, ident_sb,
                                         eu[:, cc * TP:(cc + 1) * TP],
                                         start=(cc == 0), stop=(cc == NCH - 1),
                                         skip_group_check=True)
                        nc.tensor.matmul(ps_num, ident_sb,
                                         g[:, cc * TP:(cc + 1) * TP],
                                         start=(cc == 0), stop=(cc == NCH - 1),
                                         skip_group_check=True)

                    # previous tile's softmax epilogue lands here so its
                    # reciprocal overlaps this tile's accumulation on the PE
                    if pend is not None:
                        epilogue(*pend)
                    pend = (t, ps_den, ps_num)
                epilogue(*pend)
    nc.compile()
    return nc


_NC = None


def _get_nc():
    global _NC
    if _NC is None:
        _NC = _build_kernel()
    return _NC


_PERM = (np.arange(128) + 64) % 128


def _prep_core(core, x, pos, neigh, Wc, Wn, Wr, wd, W_att, W_glob, b_rel, b_glob):
    b = core // 2
    half = core % 2
    P0 = half * NP
    nb = neigh[b][P0:P0 + NP].astype(np.int64)      # [NP, K]

    # packed gather table: row n = [x[:, n] | pos[n] | 0pad]  (bf16)
    tabT = np.zeros((N, 128), dtype=BF)
    tabT[:, 0:64] = x[b].T.astype(BF)
    tabT[:, 64:67] = pos[b].astype(BF)

    # gather idx: tile t cols (k,i) -> nb[t*512+i, k]; wrapped 16 + replicated
    A = nb.reshape(NT, TP, K)                        # [t, i, k]
    V = A.transpose(0, 2, 1).reshape(NT, PKT)        # [t, col] col=k*512+i
    W16 = V.reshape(NT, TP, 16).transpose(0, 2, 1)   # [t, j, s]: idx s*16+j
    idxg = np.tile(W16.transpose(1, 0, 2).reshape(16, NP), (8, 1)).astype(np.int16)

    posCa = pos[b][P0:P0 + NP].T.astype(BF)          # [3, NP]

    w7v = np.zeros((128, 64), dtype=BF)
    w7v[64:67] = (Wn + Wr).astype(BF)
    w7v[67:68] = wd.astype(BF)
    w7v[68:71] = (Wc - Wr).astype(BF)

    w48 = np.zeros((128, 16), dtype=BF)
    for j in range(3):
        for k in range(16):
            w48[16 * j + k, k] = 1

    brel_full = np.zeros((128, 1), np.float32)
    brel_full[64:128, 0] = b_rel

    return {
        "tabT": tabT, "idxg": idxg, "posC": posCa,
        "w7": w7v,
        "watt": W_att[np.ix_(_PERM, _PERM)].astype(BF),
        "wglob": W_glob[_PERM, :].astype(BF),
        "w48": w48,
        "ident": np.eye(128, dtype=BF),
        "brel": brel_full,
        "bglob": b_glob.reshape(128, 1).astype(np.float32),
    }


def kernel(x, pos, neigh_idx, W_rel, b_rel, W_att, W_glob, b_glob, **kw):
    x = np.ascontiguousarray(np.asarray(x, dtype=np.float32))
    pos = np.ascontiguousarray(np.asarray(pos, dtype=np.float32))
    neigh = np.asarray(neigh_idx)
    W_rel = np.asarray(W_rel, dtype=np.float32)
    W_att = np.asarray(W_att, dtype=np.float32)
    W_glob = np.asarray(W_glob, dtype=np.float32)
    b_rel = np.asarray(b_rel, dtype=np.float32)
    b_glob = np.asarray(b_glob, dtype=np.float32)
    Wc, Wn, Wr, wd = W_rel[0:3], W_rel[3:6], W_rel[6:9], W_rel[9:10]

    nc = _get_nc()
    in_maps = [
        _prep_core(core, x, pos, neigh, Wc, Wn, Wr, wd, W_att, W_glob, b_rel, b_glob)
        for core in range(8)
    ]
    res = run_bass_kernel_spmd(nc, in_maps, core_ids=list(range(8)))
    out = np.zeros((B, C_OUT, N), np.float32)
    for core in range(8):
        b = core // 2
        P0 = (core % 2) * NP
        out[b, :, P0:P0 + NP] = res.results[core]["outp"]
    return out
